# revision 47
# baseline (speedup 1.0000x reference)
"""Trainium2 Bass kernel for nn_CIN (Compressed Interaction Network).

Math (per layer k, x0 = x fixed):
    x_{k+1}[b,h,d] = sum_{i,j} W[i,j,h] * x0[b,i,d] * xk[b,j,d]
    outs_k[b,h]    = sum_d x_{k+1}[b,h,d]
    output = concat(outs_0, outs_1, outs_2)   # [B, 384]

Strategy (pure data parallel over batch, 8 cores x 128 batches):
  - bf16 compute, fp32 PSUM accumulation.
  - Per core, 8 blocks of 16 batches; free dim F = 16*64 = 1024 (b,d).
  - Layer 0 uses the i<=j symmetry: 820 unique pairs, W0sym = W0[i,j]+W0[j,i]
    (host-folded), pair products built from two host-prepared gather images
    (XSUF = x[j(c)], RSUF = x[i(c)]) stored partition-major so every DMA
    descriptor is a fat contiguous per-partition run (8 KB), not 2 KB strided.
  - Layer 1 products P[(i,j), f] = x0[i,f]*x1[j,f]: REP_i tiles (x0 row i
    broadcast across partitions) via DMA stride-0 partition-broadcast APs,
    4 chunks per DMA, alternating the two HWDGE rings; products via fused
    group tensor_tensor (one instr = 4 chunks, stride-0 middle dim on the
    x1 operand keeps the DVE 2x bf16 mode and amortizes overhead 4x).
  - Matmuls: stationary = W chunk [c,h], moving = P chunk [c, 512] (PSUM
    write cap), emitted 512-half-first in layer 0 so the x1 copy / first
    L1 product chain overlaps the second half's matmuls. PSUM accumulation
    -> x_{k+1} in [h, (b,d)] layout = next layer's input layout.
  - Layer 2 never materializes x3: outs_2 = W2 : G2 where
    G2'[b][j,i] = sum_d x2[b,j,d]*x0[b,i,d] (small per-batch Gram via PE),
    then one 40-chunk contraction; a ones-column appended to xdt makes the
    gram emit outs_1 for free. Saves 43% of FLOPs and a third of the
    elementwise work.
  - Steady state is DMA-byte-bound (~12.3MB/block through 16 queues at
    ~23GB/s each): 2 of 10 REP groups are built on the PE (rank-1
    ones-matmul + ACT copy) to shed DMA bytes, their x0 sources loaded
    ahead of the broadcast queue so the ones-matmuls fill the L0->L1
    boundary and keep the HAM clock gate warm. A 32-matmul warm-up spin
    covers the DMA-only startup window.
"""
import os
import sys

sys.path.insert(0, "/opt/trn_rl_repo")
os.environ.setdefault("JAX_PLATFORMS", "cpu")

from contextlib import ExitStack

import numpy as np
import ml_dtypes

import concourse.bass as bass  # noqa: F401
import concourse.tile as tile
from concourse import bacc, library_config, mybir
from concourse.bass_utils import run_bass_kernel_spmd

BF16 = mybir.dt.bfloat16
F32 = mybir.dt.float32
NPBF16 = ml_dtypes.bfloat16

B, M, D, HK = 1024, 40, 64, 128
NCORE = 8
BS = B // NCORE          # 128 batches per core
NBLK = 8                 # blocks per core
BB = BS // NBLK          # 16 batches per block
F = BB * D               # 1024 free elements per block
NI = M                   # 40 chunks in layer 1
GRP = 4                  # chunks fused per group (one TT / one REP DMA)
NGRP = NI // GRP         # 10 groups in layer 1
NP0 = M * (M + 1) // 2   # 820 unique layer-0 pairs
NCH0 = 7                 # layer-0 chunks (820 -> 896 rows, 76 zero-pad)
GRP0S = (4, 3)           # layer-0 group sizes (4 + 3 chunks)
NGRP0 = len(GRP0S)
NMM = F // 512           # matmuls per chunk (PSUM write = 512 els max)
HB = BB // 2             # L2 gram batch (8 transposes/grams per PSUM tile)
PE_REP = 2               # trailing REP groups built by PE ones-matmul

_PROFILE = False
_TRACE_KW = {}
_nc_cache = None
_last_results = None


def _build():
    nc = bacc.Bacc("TRN2", target_bir_lowering=False, debug=False,
                   enable_asserts=False)

    xsuf_d = nc.dram_tensor("xsuf", [NBLK, 128, NCH0, F], BF16,
                            kind="ExternalInput").ap()
    rsuf_d = nc.dram_tensor("rsuf", [NBLK, 128, NCH0, F], BF16,
                            kind="ExternalInput").ap()
    x0f_d = nc.dram_tensor("x0f", [NBLK, NI, F], BF16, kind="ExternalInput").ap()
    xdt_d = nc.dram_tensor("xdt", [D, BS, M + 1], BF16, kind="ExternalInput").ap()
    w0_d = nc.dram_tensor("w0", [128, NCH0, HK], BF16, kind="ExternalInput").ap()
    w1_d = nc.dram_tensor("w1", [128, NI, HK], BF16, kind="ExternalInput").ap()
    w2_d = nc.dram_tensor("w2", [128, NI, HK], BF16, kind="ExternalInput").ap()
    idb_d = nc.dram_tensor("idb", [128, 128], BF16, kind="ExternalInput").ap()
    idf_d = nc.dram_tensor("idf", [128, 128], F32, kind="ExternalInput").ap()
    out_d = nc.dram_tensor("out", [BS, 3 * HK], F32, kind="ExternalOutput").ap()

    with tile.TileContext(nc) as tc, ExitStack() as ctx:
        stat = ctx.enter_context(tc.tile_pool(name="stat", bufs=1))
        sufp = ctx.enter_context(tc.tile_pool(name="sufp", bufs=4))
        repp = ctx.enter_context(tc.tile_pool(name="repp", bufs=6))
        x0pp = ctx.enter_context(tc.tile_pool(name="x0pp", bufs=3))
        pp = ctx.enter_context(tc.tile_pool(name="pp", bufs=2))
        xkp = ctx.enter_context(tc.tile_pool(name="xkp", bufs=4))
        x2tp = ctx.enter_context(tc.tile_pool(name="x2tp", bufs=2))
        ps_acc = ctx.enter_context(tc.tile_pool(name="ps_acc", bufs=2, space="PSUM"))
        ps_tr = ctx.enter_context(tc.tile_pool(name="ps_tr", bufs=1, space="PSUM"))
        ps_sm = ctx.enter_context(tc.tile_pool(name="ps_sm", bufs=1, space="PSUM"))
        ps_rp = ctx.enter_context(tc.tile_pool(name="ps_rp", bufs=2, space="PSUM"))

        # critical loads first (w0/w1 are fat contiguous descriptors now);
        # remaining static tensors stream in behind block-0/1 traffic
        w0sb = stat.tile([128, NCH0, HK], BF16, tag="w0sb")
        nc.sync.dma_start(w0sb[:], w0_d[:])
        w1sb = stat.tile([128, NI, HK], BF16, tag="w1sb")
        w2sb = stat.tile([128, NI, HK], BF16, tag="w2sb")
        xdt_sb = stat.tile([D, BS, M + 1], BF16, tag="xdt_sb")
        idb = stat.tile([128, 128], BF16, tag="idb")
        idf = stat.tile([128, 128], F32, tag="idf")
        g2stack = stat.tile([128, NI, BS], BF16, tag="g2stack")
        outs_sb = stat.tile([128, 3, BS], F32, tag="outs_sb")
        outT_sb = stat.tile([128, 3, HK], F32, tag="outT_sb")
        ones_sb = stat.tile([1, 512], BF16, tag="ones_sb")
        nc.vector.memset(ones_sb[:], 1.0)

        # PE warm-up spin: the HAM clock gate starts at 1.2GHz and needs
        # ~3.4us of sustained activity to reach 2.4GHz. The first ~20us of
        # the kernel is DMA-only (startup loads), so burn it on dummy
        # rank-1 matmuls; block 0 then starts at full clock.
        warm_ps = ps_rp.tile([128, 512], F32, tag="rp")
        for _ in range(32):
            nc.tensor.matmul(warm_ps[:], ones_sb[:, 0:128], ones_sb[:],
                             start=True, stop=True)

        def emit_l2(pblk, px2sb):
            """Layer-2 grams for block `pblk` (software-pipelined one block
            late so its PE ops fill the L0/L1 dependency gaps of the next
            block). Batched: HB transposes -> one copy -> HB grams -> one copy.
            G2'[b][j,i] = sum_d x2[b,j,d]*x0[b,i,d]. The xdt ones-column
            makes the gram also produce outs_1[b,j] = sum_d x2[b,j,d] for
            free (no DVE reduce needed for layer 1)."""
            for h in range(BB // HB):
                x2t_ps = ps_tr.tile([D, HB, 128], BF16, tag="x2t")
                for e in range(HB):
                    b8 = h * HB + e
                    nc.tensor.transpose(x2t_ps[:, e, :],
                                        px2sb[:, b8 * D:(b8 + 1) * D], idb[:])
                x2t = x2tp.tile([D, HB, 128], BF16, tag="x2t_sb")
                nc.scalar.copy(x2t[:], x2t_ps[:])
                g2ps = ps_sm.tile([128, HB, NI + 1], F32, tag="sm")
                for e in range(HB):
                    b = pblk * BB + h * HB + e
                    nc.tensor.matmul(g2ps[:, e, :], x2t[:, e, :],
                                     xdt_sb[:, b, :], start=True, stop=True)
                b0 = pblk * BB + h * HB
                nc.scalar.copy(
                    g2stack[:, :, b0:b0 + HB],
                    g2ps[:, :, :NI].rearrange("p e i -> p i e"))
                nc.scalar.copy(outs_sb[:, 1, b0:b0 + HB], g2ps[:, :, NI])

        x2sb_prev = None
        for blk in range(NBLK):
            # ---- layer 0 (symmetric pairs) ----
            # Matmuls emitted 512-column-half first (s-outer) so the first
            # PSUM half finishes early; its x1 copy and the first L1 product
            # then overlap the second half's matmuls instead of serializing.
            x1ps = ps_acc.tile([128, F], F32, tag="acc")
            p0_tiles = []
            ch = 0
            for g, gsz in enumerate(GRP0S):
                xs_t = sufp.tile([128, gsz, F], BF16, tag="suf")
                nc.sync.dma_start(xs_t[:], xsuf_d[blk, :, ch:ch + gsz, :])
                rs_t = sufp.tile([128, gsz, F], BF16, tag="suf")
                nc.scalar.dma_start(rs_t[:], rsuf_d[blk, :, ch:ch + gsz, :])
                p_t = pp.tile([128, GRP, F], BF16, tag="p")
                nc.vector.tensor_mul(p_t[:, :gsz, :], xs_t[:], rs_t[:])
                p0_tiles.append((ch, gsz, p_t))
                ch += gsz
            x1sb = xkp.tile([128, F], BF16, tag="xk")
            for s in range(NMM):
                for (c0, gsz, p_t) in p0_tiles:
                    for e in range(gsz):
                        nc.tensor.matmul(
                            x1ps[:, s * 512:(s + 1) * 512],
                            w0sb[:, c0 + e, :],
                            p_t[:, e, s * 512:(s + 1) * 512],
                            start=(c0 + e == 0), stop=(c0 + e == NCH0 - 1))
                nc.scalar.copy(x1sb[:, s * 512:(s + 1) * 512],
                               x1ps[:, s * 512:(s + 1) * 512])

            if blk == 0:
                # behind block-0's rsuf loads on the scalar ring: the first
                # products aren't stuck behind this 1.25MB transfer, and it
                # still lands well before L1(0) needs it
                nc.scalar.dma_start(w1sb[:], w1_d[:])
                nc.scalar.dma_start(idb[:], idb_d[:])
            elif blk == 1:
                nc.scalar.dma_start(xdt_sb[:], xdt_d[:])
                nc.sync.dma_start(w2sb[:], w2_d[:])
                nc.sync.dma_start(idf[:], idf_d[:])

            # previous block's layer-2 PE work fills the x1-copy/TT gap
            # (emitted after the x1 copies so the ACT queue serves those first)
            if x2sb_prev is not None:
                emit_l2(blk - 1, x2sb_prev[:])

            # REP groups for layer 1: x0 rows broadcast across partitions.
            # Most groups via stride-0 DMA; `pe_set` groups via PE ones-matmul
            # (rank-1 broadcast) + ACT PSUM copies, offloading the saturated
            # DMA rings.
            pe_set = set(range(NGRP - PE_REP, NGRP))

            def gen_pe_rep(g, rg, x0pe, cnt=[0]):
                for e in range(GRP):
                    for s in range(NMM):
                        st = ps_rp.tile([128, 512], F32, tag="rp")
                        nc.tensor.matmul(
                            st[:], ones_sb[:, 0:128],
                            x0pe[0:1, e, s * 512:(s + 1) * 512],
                            start=True, stop=True)
                        nc.scalar.copy(
                            rg[:, e, s * 512:(s + 1) * 512], st[:])

            # x0pe loads first: they are tiny and must land before the rep
            # broadcasts queue up ~7MB on the rings, so the ones-matmuls can
            # fill the L0->L1 boundary (and keep the PE clock-gate warm)
            x0pe_t = {}
            for g in sorted(pe_set):
                x0pe = x0pp.tile([1, GRP, F], BF16, tag="x0pe")
                nc.sync.dma_start(
                    x0pe[:], x0f_d[blk:blk + 1, g * GRP:(g + 1) * GRP, :])
                x0pe_t[g] = x0pe
            rep_grps = {}
            for g in range(NGRP):
                rg = repp.tile([128, GRP, F], BF16, tag="rep")
                if g not in pe_set:
                    eng = nc.sync if g % 2 == 0 else nc.scalar
                    eng.dma_start(
                        rg[:], x0f_d[blk:blk + 1, g * GRP:(g + 1) * GRP, :]
                        .partition_broadcast(128))
                rep_grps[g] = rg
            for g in sorted(pe_set):
                gen_pe_rep(g, rep_grps[g], x0pe_t[g])

            # ---- layer 1 ----
            x2ps = ps_acc.tile([128, F], F32, tag="acc")
            x1b = x1sb[:].unsqueeze(1).broadcast_to([128, GRP, F])
            for g in range(NGRP):
                p_t = pp.tile([128, GRP, F], BF16, tag="p")
                nc.vector.tensor_mul(p_t[:], x1b, rep_grps[g][:])
                for e in range(GRP):
                    i = g * GRP + e
                    for s in range(NMM):
                        nc.tensor.matmul(
                            x2ps[:, s * 512:(s + 1) * 512],
                            w1sb[:, i, :],
                            p_t[:, e, s * 512:(s + 1) * 512],
                            start=(i == 0), stop=(i == NI - 1))
            # outs_0 reduce emitted late so the DVE queue serves products first
            nc.vector.tensor_reduce(
                outs_sb[:, 0, blk * BB:(blk + 1) * BB],
                x1sb[:].rearrange("p (b d) -> p b d", d=D),
                axis=mybir.AxisListType.X, op=mybir.AluOpType.add)
            x2sb = xkp.tile([128, F], BF16, tag="xk")
            nc.scalar.copy(x2sb[:], x2ps[:])
            x2sb_prev = x2sb

        emit_l2(NBLK - 1, x2sb_prev[:])

        # ---- outs_2 = W2 : G2 ----
        out2ps = ps_sm.tile([HK, BS], F32, tag="sm")
        for i in range(NI):
            nc.tensor.matmul(out2ps[:], w2sb[:, i, :], g2stack[:, i, :],
                             start=(i == 0), stop=(i == NI - 1))
        nc.scalar.copy(outs_sb[:, 2, :], out2ps[:])

        # ---- transpose [h, b] -> [b, h] and store ----
        for k in range(3):
            trp = ps_sm.tile([128, 128], F32, tag="sm")
            nc.tensor.transpose(trp[:], outs_sb[:, k, :], idf[:])
            nc.scalar.copy(outT_sb[:, k, :], trp[:])
        nc.sync.dma_start(out_d[:], outT_sb[:])

    nc.compile()
    return nc


_II0, _JJ0 = np.triu_indices(M)          # 820 pairs, i <= j


def _host_prep(x, W0, W1, W2):
    """Build per-core input maps. All reshapes/casts in numpy."""
    # layer-0 symmetric weights: W0s[c,h] = W0[i,j,h] + W0[j,i,h] (i<j), diag 1x
    w0sym = W0[_II0, _JJ0, :] + np.where(
        (_II0 != _JJ0)[:, None], W0[_JJ0, _II0, :], 0.0)          # [820, HK]
    w0pad = np.zeros((NCH0 * 128, HK), np.float32)
    w0pad[:NP0] = w0sym
    # partition-major: [128, NCH0, HK] so the load is one fat run per partition
    w0p = np.ascontiguousarray(
        w0pad.reshape(NCH0, 128, HK).transpose(1, 0, 2)).astype(NPBF16)
    w1t = np.ascontiguousarray(W1.transpose(1, 0, 2)).astype(NPBF16)
    w2t = np.ascontiguousarray(W2.transpose(1, 0, 2)).astype(NPBF16)
    idb = np.eye(128, dtype=np.float32).astype(NPBF16)
    idf = np.eye(128, dtype=np.float32)

    # padded pair index maps (pad rows point at row 0 but weights are zero;
    # use an explicit zero row instead to keep P small and exact)
    ii = np.zeros(NCH0 * 128, np.int64)
    jj = np.zeros(NCH0 * 128, np.int64)
    ii[:NP0] = _II0
    jj[:NP0] = _JJ0
    pad_mask = np.zeros((NCH0 * 128, 1), np.float32)
    pad_mask[:NP0] = 1.0

    xbf = x.astype(NPBF16)
    in_maps = []
    for c in range(NCORE):
        xs = xbf[c * BS:(c + 1) * BS]                     # [BS, M, D]
        xsT = xs.transpose(1, 0, 2)                       # [M, BS, D]
        xf = xsT.reshape(M, NBLK, F).astype(np.float32)   # [M, NBLK, F]
        x0f = np.ascontiguousarray(
            xf.transpose(1, 0, 2)).astype(NPBF16)         # [NBLK, M, F]
        # gather images for layer-0 pairs, partition-major:
        # [NCH0*128 rows, NBLK, F] -> [NBLK, 128, NCH0, F]
        xsuf = (xf[jj] * pad_mask[:, :, None]).transpose(1, 0, 2)
        rsuf = (xf[ii] * pad_mask[:, :, None]).transpose(1, 0, 2)
        xsuf = np.ascontiguousarray(
            xsuf.reshape(NBLK, NCH0, 128, F).transpose(0, 2, 1, 3)
        ).astype(NPBF16)
        rsuf = np.ascontiguousarray(
            rsuf.reshape(NBLK, NCH0, 128, F).transpose(0, 2, 1, 3)
        ).astype(NPBF16)
        # [D, BS, M+1]: trailing ones column makes the L2 gram emit outs_1
        xdt = np.concatenate(
            [xs.transpose(2, 0, 1),
             np.ones((D, BS, 1), NPBF16)], axis=2)
        xdt = np.ascontiguousarray(xdt)
        in_maps.append({
            "xsuf": xsuf, "rsuf": rsuf, "x0f": x0f, "xdt": xdt,
            "w0": w0p, "w1": w1t, "w2": w2t,
            "idb": idb, "idf": idf,
        })
    return in_maps


def kernel(x, W0, W1, W2):
    global _nc_cache, _last_results
    x = np.asarray(x, dtype=np.float32)
    W0 = np.asarray(W0, dtype=np.float32)
    W1 = np.asarray(W1, dtype=np.float32)
    W2 = np.asarray(W2, dtype=np.float32)

    if _nc_cache is None:
        _nc_cache = _build()
    nc = _nc_cache

    in_maps = _host_prep(x, W0, W1, W2)
    res = run_bass_kernel_spmd(nc, in_maps, list(range(NCORE)),
                               trace=_PROFILE, **_TRACE_KW)
    _last_results = res
    out = np.concatenate(
        [np.asarray(res.results[c]["out"]) for c in range(NCORE)], axis=0)
    return out.astype(np.float32)
